# revision 35
# baseline (speedup 1.0000x reference)
"""Trainium2 Bass kernel for nn_AttentionBlock (sparse attention block).

Reference computation (B=4, C=512, T=2048, H=8 heads, 32 GN groups):
    xn  = GroupNorm(x) * gn_w + gn_b
    qkv = qkv_w @ xn + qkv_b            (1x1 conv)
    q,k,v = split(reshape(qkv, [B*H, 192, T])) ; each += pos
    S   = (q*s)^T (k*s),  s = ch^-0.25  => scale 1/8 on logits
    S[mask keys] = -1e9 ; P = softmax(S, axis=keys)
    h   = P @ v ; out = x + proj_w @ h + proj_b

Mask quirk (faithful to the reference): jnp.tile(mask,(H,1,1)) tiles
head-major, so attention row n = b*H + h uses mask[n % B] = mask[h % 4] —
every batch's head h is masked with mask[h mod 4], not its own batch mask.

Sharding: 8 cores = (batch b, query-half j).  Each core computes
out[b][:, j*1024:(j+1)*1024] completely; host concatenates.  No collectives.

Sparsity: host compacts the key axis per mask-group m = h%4 with
keep_m = ~mask[m] (about half of T), padded to a common multiple of 128.
Padded key rows get an exp-bias of -1e9 so they contribute exactly 0.

Head layout on device: slot order [0,4,1,5,2,6,3,7] so the two heads of a
mask-group (m, m+4) sit in one 128-partition pair; host reorders the qkv
weights / biases / pos / proj rows to match, so the device never permutes.

Device layout tricks: scores are computed transposed, S^T [keys, queries]:
  - the pad bias is per-partition and folds into the ACT exp for free,
  - the softmax denominator comes from an extra ones-column appended to V^T
    during the PV matmul (row 64 of the PV psum accumulates sum_s exp(S)).
GroupNorm statistics are folded on the host into a per-channel affine (A, B)
so the device applies xn = x*A + B with one tensor_scalar op per tile.

Performance structure (what made it fast):
  - proj contracts four stacked head-pair tiles [128, T] so every proj
    pass uses the full 128-row contraction (2x over per-head 64 rows).
  - normalize is PE-free: ACT Copy moves the denominator row to
    partition 0 (Copy is in every ACT table set - no table reload),
    GPSIMD partition_broadcast spreads it over 64 rows, a 64-lane
    reciprocal_approx_fast inverts, DVE multiplies into h_pair.  Chains
    are per-(head, 512-col block) and hide under neighboring matmuls.
  - attention pipelines the PV matmuls one key-chunk behind S/exp, so
    the PE never waits on the ACT exp; the last group runs as two
    column halves so its normalize hides under the other half / proj.
  - all v-projections run first (their PE work covers the input DMA
    stream), DMAs are emitted in first-use order on two issue engines,
    and pad/gnAB are host-packed partition-major (tiny-element DMAs
    cost microseconds of descriptor issue otherwise).
  - fp8/DoubleRow was tried and rejected: logits here reach 47 with
    softmax N_eff ~ 8, so fp8 quantization noise on q/k/v does not
    average out (measured 2.9e-2 rel err), and 128-column DR matmuls
    are LDWEIGHTS-bound (335 ns for 107 ns of work).
"""

import numpy as np
import ml_dtypes

B, C, T, H = 4, 512, 2048, 8
CH = C // H          # 64 channels per head
TH = T // 2          # 1024 query columns per core
P = 128
NUM_GROUPS = 32
GS = C // NUM_GROUPS  # 16 channels per group
EPS = 1e-5
BF16 = ml_dtypes.bfloat16
NMG = 4              # mask groups (= B); group m covers heads m and m+4
PERM = [0, 4, 1, 5, 2, 6, 3, 7]  # slot s holds true head PERM[s]

_graph_cache = {}


def _build(nkv, scs):
    """Build the Bass graph for one core (SPMD: all 8 cores run this graph)."""
    import concourse.tile as tile
    from concourse import bacc, mybir

    f32 = mybir.dt.float32
    bf16 = mybir.dt.bfloat16
    AF = mybir.ActivationFunctionType
    OP = mybir.AluOpType

    sc_n = nkv // P  # max number of 128-wide key chunks (buffer sizing)
    # scs[m]: chunks actually carrying kept keys for mask-group m; chunks
    # beyond scs[m] are all-padding (zero weight) and are skipped outright.

    nc = bacc.Bacc("TRN2")

    # ---- DRAM parameters (per-core shards; host fills these) ----
    d_xq = nc.dram_tensor("x_q", [C, TH], bf16, kind="ExternalInput")
    d_xkv = nc.dram_tensor("x_kv", [NMG, C, nkv], bf16, kind="ExternalInput")
    d_xres = nc.dram_tensor("x_res", [C, TH], f32, kind="ExternalInput")
    d_AB = nc.dram_tensor("gn_AB", [P, 8], f32, kind="ExternalInput")
    d_posq = nc.dram_tensor("pos_q", [C, TH], bf16, kind="ExternalInput")
    d_poskv = nc.dram_tensor("pos_kv", [NMG, P, nkv], bf16, kind="ExternalInput")
    d_posvT = nc.dram_tensor("posvT", [NMG, P, nkv], bf16,
                             kind="ExternalInput")
    d_wqkvT = nc.dram_tensor("wqkvT", [C, 3 * C], bf16, kind="ExternalInput")
    d_wpT = nc.dram_tensor("wpT", [C, C], bf16, kind="ExternalInput")
    # pad/gnAB come pre-packed partition-major so their DMAs are one
    # contiguous run per partition (tiny-element DMAs cost µs to issue).
    d_pad = nc.dram_tensor("pad_bias", [NMG, P, nkv // P], f32,
                           kind="ExternalInput")
    d_out = nc.dram_tensor("out", [C, TH], f32, kind="ExternalOutput")

    with tile.TileContext(nc) as tc, \
         tc.tile_pool(name="persist", bufs=1) as pers, \
         tc.tile_pool(name="mm", bufs=2, space="PSUM") as mmp, \
         tc.tile_pool(name="opool", bufs=4, space="PSUM") as opl, \
         tc.tile_pool(name="exps", bufs=6) as epl, \
         tc.tile_pool(name="misc", bufs=2) as msc, \
         tc.tile_pool(name="nrm", bufs=1) as nrm:

        def ptile(shape, dt_, name):
            return pers.tile(shape, dt_, tag=name, name=name)

        # --- tiny exp to pull the ACT table load off the critical path ---
        warm_in = ptile([1, 1], f32, "warm_in")
        warm_out = ptile([1, 1], f32, "warm_out")
        nc.vector.memset(warm_in, 0.0)
        nc.scalar.activation(out=warm_out, in_=warm_in, func=AF.Exp)

        # --- persistent SBUF arrays (combined tiles = fewer, bigger DMAs) ---
        # xkv[m]: [128, 4 chan-blocks * nkv]; block i = channels [128i,128i+128)
        xkv = [ptile([P, 4 * nkv], bf16, f"xkv{m}") for m in range(NMG)]
        # vhat[m]: [128, sc_n*192]; chunk s: cols [192s,192s+128) hold the
        # two heads' v^T channel-INTERLEAVED ([a0 b0 a1 b1 ...], written by
        # one XBAR dma-transpose, 128B-aligned), cols 192s+128/129 hold the
        # softmax ones; head j reads [65] via a stride-2 AP view.
        vhat = [ptile([P, sc_n * 192], bf16, f"vhat{m}") for m in range(NMG)]
        posvT = [ptile([P, nkv], bf16, f"posvT{m}") for m in range(NMG)]
        pad_sb = [ptile([P, sc_n], f32, f"pad{m}") for m in range(NMG)]
        gnAB = ptile([P, 4 * 2], f32, "gnAB")       # block i = [A_i, B_i]
        wv = ptile([P, 4 * C], bf16, "wv")          # block i = v-rows of W^T
        wqk = ptile([P, 4 * 2 * C], bf16, "wqk")    # block i = [q|k]-rows
        xq = ptile([P, 4 * TH], bf16, "xq")
        posq = ptile([P, 4 * TH], bf16, "posq")
        poskv = [ptile([P, nkv], bf16, f"poskv{m}") for m in range(NMG)]
        xres = ptile([P, 4 * TH], f32, "xres")
        wp = ptile([P, 4 * C], bf16, "wp")          # block pm = proj row pair
        q_sb = [ptile([P, TH], bf16, f"q{i}") for i in range(4)]
        k_sb = [ptile([P, nkv], bf16, f"k{m}") for m in range(NMG)]
        # attention output, two head slots stacked per group pair
        h_pair = [ptile([P, TH], bf16, f"h{m}") for m in range(NMG)]

        # --- input DMAs, ordered by first use; round-robin the issuing
        # engine (each dma_start costs ~0.7µs of issue time on its queue).
        _eng = [nc.sync, nc.scalar]
        _ei = [0]

        def dma(dst, src):
            _eng[_ei[0] % 2].dma_start(dst, src)
            _ei[0] += 1

        def blk_dma(dst, src, nblk):
            # src [nblk*P, W] dram -> dst [P, nblk*W] sbuf, one DMA per
            # row-block i so the issues spread across engines/queues.
            w = src.shape[-1]
            pp_ = dst.shape[0]
            for i in range(nblk):
                dma(dst[:, i * w:(i + 1) * w], src[i * pp_:(i + 1) * pp_, :])

        def dma_m_inputs(m):
            nkm = scs[m] * P
            dma(posvT[m][:, 0:nkm], d_posvT[m, :, 0:nkm])
            dma(pad_sb[m][:, 0:scs[m]],
                d_pad[m, :, 0:scs[m]])
            for i in range(4):
                dma(xkv[m][:, i * nkv:i * nkv + nkm],
                    d_xkv[m, i * P:(i + 1) * P, 0:nkm])

        dma(gnAB, d_AB[:, :])
        blk_dma(wv, d_wqkvT[:, 2 * C:3 * C], 4)
        dma_m_inputs(0)
        dma_m_inputs(1)
        dma_m_inputs(2)
        dma_m_inputs(3)
        blk_dma(wqk, d_wqkvT[:, 0:2 * C], 4)
        blk_dma(xq, d_xq[:, :], 4)
        blk_dma(posq, d_posq[:, :], 4)
        for m in range(NMG):
            dma(poskv[m][:, 0:scs[m] * P], d_poskv[m, :, 0:scs[m] * P])
        blk_dma(wp, d_wpT[:, :], 4)
        blk_dma(xres, d_xres[:, :], 4)

        # --- GroupNorm as per-channel affine (host-computed A, B) ---
        def affine_kv(m):
            for i in range(4):
                sl = xkv[m][:, i * nkv:(i + 1) * nkv]
                nc.vector.tensor_scalar(
                    out=sl, in0=sl, scalar1=gnAB[:, 2 * i:2 * i + 1],
                    scalar2=gnAB[:, 2 * i + 1:2 * i + 2],
                    op0=OP.mult, op1=OP.add)

        def emit_v(m):
            # v computed WIDE: [128 interleaved v-chans, keys] with the
            # weights stationary and keys streaming at N<=512 (0.93 ns/col
            # vs 2.4 for the old keys-stationary N=128 form).  Pos+bias is
            # added in the same orientation, then one HW XBAR dma-transpose
            # per key-chunk drops v^T into vhat (dst 128B-aligned, src at
            # partition base 0 - both XBAR requirements).
            ones_view = vhat[m].rearrange(
                "p (s c) -> p s c", c=192)[:, 0:scs[m], 128:130]
            nc.vector.memset(ones_view, 1.0)
            nkm = scs[m] * P
            vs = msc.tile([P, nkm], bf16, tag="vsb", name=f"vsb{m}")
            for st in range(0, nkm, 512):
                w_ = min(512, nkm - st)
                pvw = mmp.tile([P, 512], f32, tag="mm", name=f"psv{m}_{st}")
                for i in range(4):
                    nc.tensor.matmul(
                        pvw[:, 0:w_],
                        wv[:, i * C + m * P:i * C + (m + 1) * P],
                        xkv[m][:, i * nkv + st:i * nkv + st + w_],
                        start=(i == 0), stop=(i == 3))
                nc.vector.tensor_add(
                    vs[:, st:st + w_], pvw[:, 0:w_],
                    posvT[m][:, st:st + w_])
            for s in range(scs[m]):
                _eng[_ei[0] % 2].dma_start_transpose(
                    vhat[m][:, s * 192:s * 192 + P],
                    vs[:, s * P:(s + 1) * P])
                _ei[0] += 1

        def emit_qk(m):
            # q channels (slot order) [128*m, 128*m+128)
            pq = mmp.tile([P, TH], f32, tag="mm", name=f"psq{m}")
            for tb in range(2):
                for i in range(4):
                    nc.tensor.matmul(
                        pq[:, tb * 512:(tb + 1) * 512],
                        wqk[:, 2 * i * C + m * P:2 * i * C + (m + 1) * P],
                        xq[:, i * TH + tb * 512:i * TH + (tb + 1) * 512],
                        start=(i == 0), stop=(i == 3))
            nc.vector.tensor_add(q_sb[m], pq, posq[:, m * TH:(m + 1) * TH])
            nkm = scs[m] * P
            nb_blocks = [(st, min(512, nkm - st)) for st in range(0, nkm, 512)]
            for bi, (st, w) in enumerate(nb_blocks):
                pk = mmp.tile([P, 512], f32, tag="mm", name=f"psk{m}_{bi}")
                for i in range(4):
                    nc.tensor.matmul(
                        pk[:, 0:w],
                        wqk[:, (2 * i + 1) * C + m * P:(2 * i + 1) * C + (m + 1) * P],
                        xkv[m][:, i * nkv + st:i * nkv + st + w],
                        start=(i == 0), stop=(i == 3))
                nc.vector.tensor_add(
                    k_sb[m][:, st:st + w], pk[:, 0:w],
                    poskv[m][:, st:st + w])

        def emit_attention(m, c0, cw, post_stage1=None):
            # pair m, query columns [c0, c0+cw): S^T -> exp -> O, with the
            # O matmuls software-pipelined one s-chunk behind S/exp so the
            # PE never waits on the ACT exp at chunk boundaries.
            # O accumulators are per (head, 512-col block) tiles so the
            # normalize chains release PSUM banks in quarters.
            nb = cw // 512
            lead = 1 if nb == 2 else 2
            o_a = [opl.tile([65, 512], f32, tag="O", name=f"oa{m}_{c0}_{t}")
                   for t in range(nb)]
            o_b = [opl.tile([65, 512], f32, tag="O", name=f"ob{m}_{c0}_{t}")
                   for t in range(nb)]
            exs = {}

            def s_stage(s):
                # cw=512: one [128, 1024] tile holds both heads (halves the
                # mm-pool pressure so lead-2 fits); cw=1024: one per head.
                if nb == 1:
                    sab = mmp.tile([P, 2 * cw], f32, tag="mm",
                                   name=f"sab{m}_{c0}_{s}")
                    sv = [sab[:, 0:cw], sab[:, cw:2 * cw]]
                else:
                    sa = mmp.tile([P, cw], f32, tag="mm",
                                  name=f"sa{m}_{c0}_{s}")
                    sb_ = mmp.tile([P, cw], f32, tag="mm",
                                   name=f"sb{m}_{c0}_{s}")
                    sv = [sa, sb_]
                for t in range(nb):
                    nc.tensor.matmul(
                        sv[0][:, t * 512:(t + 1) * 512],
                        k_sb[m][0:64, s * P:(s + 1) * P],
                        q_sb[m][0:64, c0 + t * 512:c0 + (t + 1) * 512],
                        start=True, stop=True)
                for t in range(nb):
                    nc.tensor.matmul(
                        sv[1][:, t * 512:(t + 1) * 512],
                        k_sb[m][64:128, s * P:(s + 1) * P],
                        q_sb[m][64:128, c0 + t * 512:c0 + (t + 1) * 512],
                        start=True, stop=True, tile_position=(64, 0))
                ex = epl.tile([P, 2 * cw], bf16, tag="expS",
                              name=f"ex{m}_{c0}_{s}")
                nc.scalar.activation(
                    out=ex[:, 0:cw], in_=sv[0], func=AF.Exp,
                    bias=pad_sb[m][:, s:s + 1], scale=0.125)
                nc.scalar.activation(
                    out=ex[:, cw:2 * cw], in_=sv[1], func=AF.Exp,
                    bias=pad_sb[m][:, s:s + 1], scale=0.125)
                exs[s] = ex

            def o_stage(s):
                ex = exs.pop(s)
                vv = [vhat[m][:, s * 192 + j:s * 192 + j + 130].rearrange(
                    "p (c two) -> p c two", two=2)[:, :, 0:1]
                    for j in range(2)]
                for t in range(nb):
                    nc.tensor.matmul(
                        o_a[t], vv[0],
                        ex[:, t * 512:(t + 1) * 512],
                        start=(s == 0), stop=(s == scs[m] - 1))
                for t in range(nb):
                    nc.tensor.matmul(
                        o_b[t], vv[1],
                        ex[:, cw + t * 512:cw + (t + 1) * 512],
                        start=(s == 0), stop=(s == scs[m] - 1))

            for s in range(scs[m]):
                s_stage(s)
                if s == 1 and post_stage1 is not None:
                    post_stage1()
                if s >= lead:
                    o_stage(s - lead)
            for s in range(scs[m] - lead, scs[m]):
                o_stage(s)
            return o_a, o_b

        def emit_normalize(m, c0, o_a, o_b):
            # normalize: h = O[0:64] / l, l = O[64].  Entirely PE-free and
            # per-(head, col-block) so the chains pipeline: ACT copies the
            # denominator row to partition 0 (Copy is in every ACT table
            # set - no reload), GPSIMD broadcasts it over 64 rows, a
            # 64-lane fast approx reciprocal inverts, DVE multiplies.
            for j, o_ in ((0, o_a), (1, o_b)):
                for t, ot_ in enumerate(o_):
                    l_sb = nrm.tile([1, 512], f32, tag=f"l{j}{t}",
                                    name=f"l{m}_{c0}_{j}{t}")
                    if m == NMG - 1:
                        # group 3 runs while ACT is saturated with its own
                        # exps; the single-lane DVE copy is cheaper there
                        nc.vector.tensor_copy(out=l_sb, in_=ot_[64:65, :])
                    else:
                        nc.scalar.activation(
                            out=l_sb, in_=ot_[64:65, :], func=AF.Copy)
                    lb = nrm.tile([CH, 512], f32, tag=f"lb{j}{t}",
                                  name=f"lb{m}_{c0}_{j}{t}")
                    nc.gpsimd.partition_broadcast(lb, l_sb)
                    nc.vector.reciprocal_approx_fast(out=lb, in_=lb)
                    nc.vector.tensor_mul(
                        h_pair[m][j * CH:(j + 1) * CH,
                                  c0 + t * 512:c0 + (t + 1) * 512],
                        ot_[0:64, :], lb)

        # --- schedule: all v/affine first (their PE work covers the input
        # DMA stream), then qk(m), norm(m-1), att(m) interleaved; the last
        # group runs as two column halves so its normalize and the proj
        # hide each other at the tail.
        for m in range(NMG):
            affine_kv(m)
            emit_v(m)
            if m == 0:
                for i in range(4):
                    sl = xq[:, i * TH:(i + 1) * TH]
                    nc.vector.tensor_scalar(
                        out=sl, in0=sl, scalar1=gnAB[:, 2 * i:2 * i + 1],
                        scalar2=gnAB[:, 2 * i + 1:2 * i + 2],
                        op0=OP.mult, op1=OP.add)
        def norm_prev(m):
            if m - 1 not in pending:
                return None
            oab = pending.pop(m - 1)
            return lambda: emit_normalize(m - 1, 0, *oab)

        pending = {}
        for m in range(NMG - 1):
            emit_qk(m)
            pending[m] = emit_attention(m, 0, TH, post_stage1=norm_prev(m))
        m = NMG - 1
        emit_qk(m)
        oh0 = emit_attention(m, 0, 512, post_stage1=norm_prev(m))
        oh1 = emit_attention(
            m, 512, 512, post_stage1=lambda: emit_normalize(m, 0, *oh0))
        emit_normalize(m, 512, *oh1)

        # ---- proj + residual (contraction over 4 stacked pairs), per
        # 512-col block: block 0 runs while normalize(3, cols 512:) is
        # still finishing on the non-PE engines.
        for tb in range(2):
            for ci in range(4):
                pp = mmp.tile([P, 512], f32, tag="mm", name=f"pp{ci}_{tb}")
                for pm in range(4):
                    nc.tensor.matmul(
                        pp, wp[:, pm * C + ci * P:pm * C + (ci + 1) * P],
                        h_pair[pm][:, tb * 512:(tb + 1) * 512],
                        start=(pm == 0), stop=(pm == 3))
                ot = msc.tile([P, 512], f32, tag="out", name=f"ot{ci}_{tb}")
                nc.vector.tensor_add(
                    ot, pp, xres[:, ci * TH + tb * 512:ci * TH + (tb + 1) * 512])
                nc.sync.dma_start(
                    d_out[ci * P:(ci + 1) * P, tb * 512:(tb + 1) * 512], ot)

    nc.finalize()
    return nc


def _prepare(inputs):
    """Host-side shard preparation. Returns (nkv, in_maps)."""
    x = np.asarray(inputs["x"], dtype=np.float32)
    pos = np.asarray(inputs["pos"], dtype=np.float32)
    mask = np.asarray(inputs["mask"])
    gn_w = np.asarray(inputs["gn_w"], dtype=np.float32)
    gn_b = np.asarray(inputs["gn_b"], dtype=np.float32)
    qkv_w = np.asarray(inputs["qkv_w"], dtype=np.float32)
    qkv_b = np.asarray(inputs["qkv_b"], dtype=np.float32)
    proj_w = np.asarray(inputs["proj_w"], dtype=np.float32)
    proj_b = np.asarray(inputs["proj_b"], dtype=np.float32)

    # GroupNorm folded to per-channel affine per batch (stats over full T,
    # matching the reference exactly).
    xg = x.reshape(B, NUM_GROUPS, GS, T)
    mu = xg.mean(axis=(2, 3))
    var = xg.var(axis=(2, 3))
    rs = 1.0 / np.sqrt(var + EPS)
    rs_c = np.repeat(rs, GS, axis=1)
    mu_c = np.repeat(mu, GS, axis=1)
    A_all = rs_c * gn_w[None, :]
    B_all = gn_b[None, :] - mu_c * A_all

    # reorder qkv weights: reference splits rows as [h, (q|k|v), 64]; we
    # additionally permute heads into slot order PERM.
    perm = np.asarray(PERM)
    w3 = qkv_w.reshape(H, 3, CH, C)
    b3 = qkv_b.reshape(H, 3, CH)
    wq_r = w3[perm, 0].reshape(C, C)
    wk_r = w3[perm, 1].reshape(C, C)
    wv_r = w3[perm, 2].reshape(C, C)
    bq = b3[perm, 0].reshape(C)
    bk = b3[perm, 1].reshape(C)
    bv = b3[perm, 2].reshape(C)
    # device weight layout: wqkvT[:, 0:2C] = interleaved [q|k] per... actually
    # [q rows | k rows | v rows] transposed, same as before.
    # v out-channels interleaved within each group-pair: row m*128+2c+j =
    # head-j channel c, matching the XBAR-transposed vhat layout.
    iv = np.array([(2 * m + j) * CH + c
                   for m in range(NMG) for c in range(CH) for j in range(2)])
    wqkvT = np.ascontiguousarray(
        np.concatenate([wq_r, wk_r, wv_r[iv]], axis=0).T).astype(BF16)
    # proj: input channels permuted to slot order
    perm_idx = (perm[:, None] * CH + np.arange(CH)[None, :]).reshape(-1)
    wpT = np.ascontiguousarray(proj_w.T[perm_idx]).astype(BF16)

    # per mask-group key compaction (mask quirk: group m uses mask[m])
    keep = [np.flatnonzero(~mask[m, 0]) for m in range(NMG)]
    n_max = max(max(len(kp) for kp in keep), 1)
    nkv = ((n_max + P - 1) // P) * P
    scs = tuple(max((len(kp) + P - 1) // P, 1) for kp in keep)

    x_kv_all = []      # per batch: [NMG, C, nkv]
    for bb in range(B):
        xkv_b = np.zeros((NMG, C, nkv), dtype=BF16)
        for m in range(NMG):
            kp = keep[m]
            xkv_b[m, :, :len(kp)] = x[bb][:, kp]
        x_kv_all.append(xkv_b)

    # packed partition-major: pad[m, p, s] = bias for key s*128 + p
    pad = np.zeros((NMG, nkv), dtype=np.float32)
    for m in range(NMG):
        pad[m, len(keep[m]):] = -1e9
    pad = np.ascontiguousarray(
        pad.reshape(NMG, nkv // P, P).transpose(0, 2, 1))

    in_maps = []
    for core in range(8):
        bb, half = core // 2, core % 2
        ts = slice(half * TH, (half + 1) * TH)
        posb = pos[bb * H:(bb + 1) * H]        # [8, 64, 2048] true head order

        x_q = np.ascontiguousarray(x[bb][:, ts]).astype(BF16)
        x_res = np.ascontiguousarray(
            x[bb][:, ts] + proj_b[:, None]).astype(np.float32)
        pos_q = (posb[perm][:, :, ts].reshape(C, TH) + bq[:, None]).astype(BF16)

        pos_kv = np.zeros((NMG, P, nkv), dtype=BF16)
        pos_vT = np.zeros((NMG, P, nkv), dtype=BF16)
        for m in range(NMG):
            kp = keep[m]
            nb = len(kp)
            for j, hh in enumerate((m, m + 4)):   # slots 2m, 2m+1
                sl = slice((2 * m + j) * CH, (2 * m + j + 1) * CH)
                pos_kv[m, j * CH:(j + 1) * CH, :nb] = (
                    posb[hh][:, kp] + bk[sl][:, None])
                # interleaved rows 2c+j for the wide-v orientation
                pos_vT[m, 2 * np.arange(CH) + j][:, :nb] = 0  # noop keep shape
                pos_vT[m, (2 * np.arange(CH) + j)[:, None],
                       np.arange(nb)[None, :]] = (
                    posb[hh][:, kp] + bv[sl][:, None]).astype(BF16)

        in_maps.append({
            "x_q": x_q,
            "x_kv": x_kv_all[bb],
            "x_res": x_res,
            "gn_AB": np.ascontiguousarray(
                np.stack([A_all[bb], B_all[bb]], axis=1).reshape(
                    4, P, 2).transpose(1, 0, 2).reshape(P, 8)
            ).astype(np.float32),
            "pos_q": pos_q,
            "pos_kv": pos_kv,
            "posvT": pos_vT,
            "wqkvT": wqkvT,
            "wpT": wpT,
            "pad_bias": pad,
        })
    return nkv, scs, in_maps


def kernel(**inputs):
    from concourse.bass_utils import run_bass_kernel_spmd

    nkv, scs, in_maps = _prepare(inputs)
    key = (nkv, scs)
    if key not in _graph_cache:
        _graph_cache[key] = _build(nkv, scs)
    nc = _graph_cache[key]

    res = run_bass_kernel_spmd(nc, in_maps, core_ids=list(range(8)))
    results = res.results

    out = np.empty((B, C, T), dtype=np.float32)
    for core in range(8):
        bb, half = core // 2, core % 2
        out[bb][:, half * TH:(half + 1) * TH] = results[core]["out"]
    return out
